# revision 13
# baseline (speedup 1.0000x reference)
"""Trainium2 Bass kernel for the contextual channel-attention transformer block.

Contract: kernel(**inputs) takes the FULL unsharded inputs
(x: (8,512,64,64) f32, Wq/Wk/Wv: (512,512) f32, gamma: (1,) f32) and
returns the FULL (8,512,64,64) f32 output.  Internally the batch is
data-parallel across 8 NeuronCores (one batch element per core).

Per-core algorithm (bf16 matmuls, fp32 PSUM accumulation):
  Gx   = X X^T                        (C x C spatial Gram, 128 MMs)
  M3q  = Gx Wq^T, M3k = Gx Wk^T       (32 MMs)
  |Q_c|^2 = colsum(Wq^T o M3q), |K_c|^2 = colsum(Wk^T o M3k)
  rsqrt via magic constant + 2 Newton steps (DVE only, no act tables)
  msc  = M3q o rq (Q-norm folded)     so the softmax reads G^T directly
  G^T  = Wk msc = (Q K^T)^T o rq      (16 MMs)
  cos -> col-max -> temperature -> softmax: free-axis ops on G^T[d, c]
  A^T  = Wv^T Msm^T                   (16 MMs)
  A''  = A + diag((rowsum+eps)/gamma) (residual identity, 1/f folded)
  y    = f o (A'' X)                  (128 MMs; f applied in PSUM copies)
"""

import os
import sys

for _p in ("/opt/trn_rl_repo", "/root/.axon_site/_ro/trn_rl_repo"):
    if os.path.isdir(_p) and _p not in sys.path:
        sys.path.insert(0, _p)

import ml_dtypes
import numpy as np

import concourse.bass as bass
import concourse.tile as tile
from concourse import bacc, bass_utils, mybir

# Problem constants (hardcoded; kernel.py must be self-contained).
B, C, HH, WW = 8, 512, 64, 64
N = HH * WW          # 4096 spatial positions
G = C // 128         # 4 channel groups of 128
N1 = N // 128        # 32 Gram chunks (128 spatial each)
NJ = N // 512        # 8 output chunks (512 spatial each)
EPS = 1e-6
INV_H = 4.0          # 1 / 0.25 temperature
FP32 = mybir.dt.float32
BF16 = mybir.dt.bfloat16
U32 = mybir.dt.uint32
MAGIC = 0x5F3759DF

_CACHE = {}


def _rsqrt(nc, pool, out_ap, in_ap, shape, tag):
    """out = 1/sqrt(in) via magic constant + 2 Newton iterations (DVE)."""
    MUL = mybir.AluOpType.mult
    ADD = mybir.AluOpType.add
    SHR = mybir.AluOpType.logical_shift_right
    SUB = mybir.AluOpType.subtract
    p, f = shape
    sh = pool.tile([p, f], U32, tag=f"{tag}_sh")
    nc.vector.tensor_scalar(sh[:], in_ap.bitcast(U32), 1, None, op0=SHR)
    mg = pool.tile([p, f], U32, tag=f"{tag}_mg")
    nc.vector.memset(mg[:], MAGIC)
    y = pool.tile([p, f], FP32, tag=f"{tag}_y")
    nc.vector.tensor_tensor(y[:].bitcast(U32), mg[:], sh[:], op=SUB)
    for it in range(2):
        a = pool.tile([p, f], FP32, tag=f"{tag}_a{it}")
        nc.vector.tensor_tensor(a[:], y[:], y[:], op=MUL)
        b = pool.tile([p, f], FP32, tag=f"{tag}_b{it}")
        nc.vector.scalar_tensor_tensor(b[:], in_ap, -0.5, a[:],
                                       op0=MUL, op1=MUL)
        c2 = pool.tile([p, f], FP32, tag=f"{tag}_c{it}")
        nc.vector.tensor_scalar(c2[:], b[:], 1.5, None, op0=ADD)
        dst = out_ap if it == 1 else y[:]
        nc.vector.tensor_tensor(dst, y[:], c2[:], op=MUL)


def _build_nc():
    nc = bacc.Bacc("TRN2", target_bir_lowering=False)

    xt_d = nc.dram_tensor("xt", [N, C], BF16, kind="ExternalInput")   # x^T
    xh_d = nc.dram_tensor("xh", [C, N], BF16, kind="ExternalInput")
    wqt_d = nc.dram_tensor("wqt", [C, C], BF16, kind="ExternalInput")  # Wq^T
    wkt_d = nc.dram_tensor("wkt", [C, C], BF16, kind="ExternalInput")  # Wk^T
    wvo_d = nc.dram_tensor("wvo", [C, C], BF16, kind="ExternalInput")  # Wv
    gcol_d = nc.dram_tensor("gamma_col", [128, 1], FP32, kind="ExternalInput")
    ocol_d = nc.dram_tensor("ones_col", [128, 1], BF16, kind="ExternalInput")
    orow_d = nc.dram_tensor("ones_row", [1, C], BF16, kind="ExternalInput")
    ident_d = nc.dram_tensor("ident", [128, 128], BF16, kind="ExternalInput")
    y_d = nc.dram_tensor("y", [C, N], BF16, kind="ExternalOutput")

    xt_v = xt_d.ap().rearrange("(i p) c -> p i c", p=128)    # [128, N1, C]
    xh_v = xh_d.ap().rearrange("(g p) n -> p g n", p=128)    # [128, G, N]
    wq_v = wqt_d.ap().rearrange("(g p) o -> p g o", p=128)   # [128, G, C]
    wk_v = wkt_d.ap().rearrange("(g p) o -> p g o", p=128)
    wv_v = wvo_d.ap().rearrange("(g p) o -> p g o", p=128)
    y_v = y_d.ap().rearrange("(g p) n -> p g n", p=128)

    MUL = mybir.AluOpType.mult
    ADD = mybir.AluOpType.add
    MIN = mybir.AluOpType.min
    AX = mybir.AxisListType.X
    Exp = mybir.ActivationFunctionType.Exp
    Copy = mybir.ActivationFunctionType.Copy

    with tile.TileContext(nc) as tc:
        with (
            tc.tile_pool(name="consts", bufs=1) as cpool,
            tc.tile_pool(name="weights", bufs=1) as wpool,
            tc.tile_pool(name="xt", bufs=NJ) as xtpool,
            tc.tile_pool(name="xh", bufs=NJ) as xhpool,
            tc.tile_pool(name="gram", bufs=1) as gpool,
            tc.tile_pool(name="small", bufs=2) as spool,
            tc.tile_pool(name="mid", bufs=3) as mpool,
            tc.tile_pool(name="msm", bufs=1) as msmpool,
            tc.tile_pool(name="outs", bufs=4) as opool,
        ):
            # ---- input DMAs: xt on sync (first), everything else scalar --
            xt0 = []
            for i in range(G):
                t = xtpool.tile([128, 1, C], BF16, tag="xt0", bufs=G,
                                name=f"xt0_{i}")
                nc.sync.dma_start(t[:], xt_v[:, i:i + 1, :])
                xt0.append(t)
            xt_t = []
            for jj in range(1, NJ):
                t = xtpool.tile([128, G, C], BF16, tag="xt", bufs=NJ - 1,
                                name=f"xt{jj}")
                nc.sync.dma_start(t[:], xt_v[:, jj * G:(jj + 1) * G, :])
                xt_t.append(t)

            def xt_chunk(i):
                return xt0[i][:, 0, :] if i < G else xt_t[i // G - 1][:, i % G, :]

            ones_col = cpool.tile([128, 1], BF16, tag="ones_col")
            nc.scalar.dma_start(ones_col[:], ocol_d.ap())
            ones_row = cpool.tile([1, C], BF16, tag="ones_row")
            nc.scalar.dma_start(ones_row[:], orow_d.ap())
            gamma_c = cpool.tile([128, 1], FP32, tag="gamma_c")
            nc.scalar.dma_start(gamma_c[:], gcol_d.ap())
            ident = cpool.tile([128, 128], BF16, tag="ident")
            nc.scalar.dma_start(ident[:], ident_d.ap())

            wq = wpool.tile([128, G, C], BF16, tag="wq")
            wk = wpool.tile([128, G, C], BF16, tag="wk")
            wv = wpool.tile([128, G, C], BF16, tag="wv")
            nc.scalar.dma_start(wq[:], wq_v)
            nc.scalar.dma_start(wk[:], wk_v)
            nc.scalar.dma_start(wv[:], wv_v)

            xh_t = []
            for j in range(NJ):
                t = xhpool.tile([128, G, 512], BF16, tag="xh", name=f"xh{j}")
                nc.scalar.dma_start(t[:], xh_v[:, :, j * 512:(j + 1) * 512])
                xh_t.append(t)

            # rgamma = 1/gamma (for the 1/f identity diagonal)
            rgamma = spool.tile([128, 1], FP32, tag="rgamma", bufs=1)
            nc.vector.reciprocal(rgamma[:], gamma_c[:])

            # ---- Gx = X X^T  (PSUM-accumulated over 32 spatial chunks) ---
            gx_sb = gpool.tile([128, G, C], BF16, tag="gx_sb")
            with tc.tile_pool(name="psGx", bufs=1, space="PSUM") as psGx:
                gx_ps = [psGx.tile([128, C], FP32, tag="gx", bufs=G,
                                   name=f"gx{cg}") for cg in range(G)]
                for i in range(N1):
                    lhs_t = xt_chunk(i)
                    for cg in range(G):
                        nc.tensor.matmul(gx_ps[cg][:],
                                         lhs_t[:, cg * 128:(cg + 1) * 128],
                                         lhs_t[:],
                                         start=(i == 0), stop=(i == N1 - 1))
                for cg in range(G):
                    eng = nc.scalar.copy if cg % 2 else nc.vector.tensor_copy
                    eng(gx_sb[:, cg, :], gx_ps[cg][:])

            # ---- M3q = Gx Wq^T, M3k = Gx Wk^T;  |Q_c|^2 on the fly ------
            m3q = gpool.tile([128, G, C], BF16, tag="m3q")
            m3k = gpool.tile([128, G, C], BF16, tag="m3k")
            tqs, tks = [], []
            with tc.tile_pool(name="psM3", bufs=1, space="PSUM") as psM3:
                sqq = psM3.tile([1, C], FP32, tag="sqq", name="sqq")
                for cg in range(G):
                    q_ps = psM3.tile([128, C], FP32, tag="m3q", bufs=2,
                                     name=f"m3q{cg}")
                    k_ps = psM3.tile([128, C], FP32, tag="m3k", bufs=2,
                                     name=f"m3k{cg}")
                    for g in range(G):
                        lhs = gx_sb[:, g, cg * 128:(cg + 1) * 128]
                        nc.tensor.matmul(q_ps[:], lhs, wq[:, g, :],
                                         start=(g == 0), stop=(g == G - 1))
                        nc.tensor.matmul(k_ps[:], lhs, wk[:, g, :],
                                         start=(g == 0), stop=(g == G - 1))
                    nc.vector.tensor_copy(m3q[:, cg, :], q_ps[:])
                    nc.scalar.copy(m3k[:, cg, :], k_ps[:])
                    # tq = Wq^T o M3q (row-block cg); sqq += colsum(tq)
                    tq = mpool.tile([128, C], BF16, tag="tq", bufs=2,
                                    name=f"tq{cg}")
                    nc.vector.tensor_tensor(tq[:], wq[:, cg, :],
                                            m3q[:, cg, :], op=MUL)
                    tqs.append(tq)
                    nc.tensor.matmul(sqq[:], ones_col[:], tq[:],
                                     start=(cg == 0), stop=(cg == G - 1))
                    tk = mpool.tile([128, C], BF16, tag="tk", bufs=2,
                                    name=f"tk{cg}")
                    nc.gpsimd.tensor_tensor(tk[:], wk[:, cg, :],
                                            m3k[:, cg, :], op=MUL)
                    tks.append(tk)

                # rq row (bf16, for rq-fold into m3q via broadcast matmul)
                sqq_sb = spool.tile([1, C], FP32, tag="sqq_sb")
                nc.scalar.copy(sqq_sb[:], sqq[:])
                rq_bf = spool.tile([1, C], BF16, tag="rq_bf")
                _rsqrt(nc, spool, rq_bf[:], sqq_sb[:], (1, C), "rq")

            msc = gpool.tile([128, G, C], BF16, tag="msc")
            rk_all = spool.tile([128, G], FP32, tag="rk_all", bufs=1)
            with tc.tile_pool(name="psN", bufs=1, space="PSUM") as psN:
                # |K_d|^2 columns (16 small MMs), batched rsqrt
                sqk_ps = [psN.tile([128, 1], FP32, tag="sqk", bufs=G,
                                   name=f"sqk{d}") for d in range(G)]
                for g in range(G):
                    for dg in range(G):
                        nc.tensor.matmul(sqk_ps[dg][:],
                                         tks[g][:, dg * 128:(dg + 1) * 128],
                                         ones_col[:],
                                         start=(g == 0), stop=(g == G - 1))
                bq_ps = psN.tile([128, C], FP32, tag="bq_ps", name="bq_ps")
                nc.tensor.matmul(bq_ps[:], ones_row[:, 0:128], rq_bf[:],
                                 start=True, stop=True)
                # msc = m3q o rq (reads bq_ps straight from PSUM)
                for g in range(G):
                    nc.vector.tensor_tensor(msc[:, g, :], m3q[:, g, :],
                                            bq_ps[:], op=MUL)
                sqk_sb = spool.tile([128, G], FP32, tag="sqk_sb", bufs=1)
                for dg in range(G):
                    nc.scalar.copy(sqk_sb[:, dg:dg + 1], sqk_ps[dg][:])
                _rsqrt(nc, spool, rk_all[:], sqk_sb[:], (128, G), "rk")

            msm = msmpool.tile([128, G, C], BF16, tag="msm")
            at_sb = gpool.tile([128, G, C], BF16, tag="at_sb")
            fcols = []
            with tc.tile_pool(name="psB", bufs=1, space="PSUM") as psB:
                # ---- G^T per d-group + softmax + A^T ---------------------
                at_ps = [psB.tile([128, C], FP32, tag="at", bufs=G,
                                  name=f"at{eg}") for eg in range(G)]
                for dg in range(G):
                    rk = rk_all[:, dg:dg + 1]
                    g_ps = psB.tile([128, C], FP32, tag="g_ps", bufs=2,
                                    name=f"g_ps{dg}")
                    for g in range(G):
                        nc.tensor.matmul(g_ps[:],
                                         wk[:, g, dg * 128:(dg + 1) * 128],
                                         msc[:, g, :],
                                         start=(g == 0), stop=(g == G - 1))
                    mn = spool.tile([128, 1], FP32, tag="mn")
                    nc.vector.tensor_reduce(mn[:], g_ps[:], axis=AX, op=MIN)
                    # den = 1 + eps - min_c(cos) = 1 + eps - mn * rk
                    mr = spool.tile([128, 1], FP32, tag="mr")
                    nc.vector.tensor_tensor(mr[:], mn[:], rk, op=MUL)
                    den = spool.tile([128, 1], FP32, tag="den")
                    nc.vector.tensor_scalar(den[:], mr[:], -1.0, 1.0 + EPS,
                                            op0=MUL, op1=ADD)
                    r = spool.tile([128, 1], FP32, tag="r")
                    nc.vector.reciprocal(r[:], den[:])
                    # exp((INV_H*r*rk) * gps + (1 - INV_H*r))
                    sv = spool.tile([128, 1], FP32, tag="sv")
                    nc.vector.scalar_tensor_tensor(sv[:], r[:], INV_H, rk,
                                                   op0=MUL, op1=MUL)
                    bv = spool.tile([128, 1], FP32, tag="bv")
                    nc.vector.tensor_scalar(bv[:], r[:], -INV_H, 1.0,
                                            op0=MUL, op1=ADD)
                    e = mpool.tile([128, C], BF16, tag="e")
                    se = spool.tile([128, 1], FP32, tag="se")
                    nc.scalar.activation(e[:], g_ps[:], Exp,
                                         bias=bv[:], scale=sv[:],
                                         accum_out=se[:])
                    rd = spool.tile([128, 1], FP32, tag="rd")
                    nc.vector.reciprocal(rd[:], se[:])
                    nc.vector.tensor_scalar(msm[:, dg, :], e[:], rd[:], None,
                                            op0=MUL)
                    # A^T accumulation over d
                    for eg in range(G):
                        nc.tensor.matmul(at_ps[eg][:],
                                         wv[:, dg, eg * 128:(eg + 1) * 128],
                                         msm[:, dg, :],
                                         start=(dg == 0), stop=(dg == G - 1))

                # row-L1 sums; f = gamma/(s+eps) fp32 cols; A'' diag = 1/f
                s_list = []
                for cg in range(G):
                    s_ps = psB.tile([128, 1], FP32, tag="g_ps", bufs=2,
                                    name=f"s_ps{cg}")
                    for dg in range(G):
                        nc.tensor.matmul(
                            s_ps[:],
                            msm[:, dg, cg * 128:(cg + 1) * 128],
                            ones_col[:], start=(dg == 0), stop=(dg == G - 1))
                    s_list.append(s_ps)
                for eg in range(G):
                    eng = nc.scalar.copy if eg % 2 else nc.vector.tensor_copy
                    eng(at_sb[:, eg, :], at_ps[eg][:])
                for cg in range(G):
                    seps = spool.tile([128, 1], FP32, tag="seps")
                    nc.vector.tensor_scalar(seps[:], s_list[cg][:],
                                            EPS, None, op0=ADD)
                    rs = spool.tile([128, 1], FP32, tag="rs")
                    nc.vector.reciprocal(rs[:], seps[:])
                    f = spool.tile([128, 1], FP32, tag="f", bufs=G,
                                   name=f"f{cg}")
                    nc.vector.tensor_tensor(f[:], rs[:], gamma_c[:], op=MUL)
                    fcols.append(f)
                    finv = spool.tile([128, 1], FP32, tag="finv")
                    nc.vector.tensor_tensor(finv[:], seps[:], rgamma[:],
                                            op=MUL)
                    di = spool.tile([128, 128], BF16, tag="di")
                    nc.gpsimd.tensor_scalar(di[:], ident[:], finv[:], None,
                                            op0=MUL)
                    blk = at_sb[:, cg, cg * 128:(cg + 1) * 128]
                    nc.gpsimd.tensor_tensor(blk, blk, di[:], op=ADD)

            # ---- phase 2: y = f o (A'' X), store bf16 --------------------
            with tc.tile_pool(name="ps2", bufs=1, space="PSUM") as ps2:
                for j in range(NJ):
                    for cg in range(G):
                        o_ps = ps2.tile([128, 512], FP32, tag="o_ps", bufs=8,
                                        name=f"o_ps{j}_{cg}")
                        for eg in range(G):
                            nc.tensor.matmul(
                                o_ps[:], at_sb[:, eg, cg * 128:(cg + 1) * 128],
                                xh_t[j][:, eg, :],
                                start=(eg == 0), stop=(eg == G - 1))
                        ofin = opool.tile([128, 512], BF16, tag="ofin",
                                          bufs=8, name=f"ofin{j}_{cg}")
                        k = j * G + cg
                        if k % 2:
                            nc.scalar.activation(ofin[:], o_ps[:], Copy,
                                                 scale=fcols[cg][:])
                            nc.scalar.dma_start(
                                y_v[:, cg, j * 512:(j + 1) * 512], ofin[:])
                        else:
                            nc.vector.tensor_scalar(ofin[:], o_ps[:],
                                                    fcols[cg][:], None,
                                                    op0=MUL)
                            nc.sync.dma_start(
                                y_v[:, cg, j * 512:(j + 1) * 512], ofin[:])

    nc.compile()
    return nc


def _get_nc():
    if "nc" not in _CACHE:
        _CACHE["nc"] = _build_nc()
    return _CACHE["nc"]


def _make_in_maps(x, Wq, Wk, Wv, gamma):
    xb_h = np.ascontiguousarray(
        x.reshape(B, C, N)).astype(ml_dtypes.bfloat16)
    xt_h = np.ascontiguousarray(xb_h.transpose(0, 2, 1))
    wqt = np.ascontiguousarray(Wq.T).astype(ml_dtypes.bfloat16)
    wkt = np.ascontiguousarray(Wk.T).astype(ml_dtypes.bfloat16)
    wvo = np.ascontiguousarray(Wv).astype(ml_dtypes.bfloat16)
    gc = np.full((128, 1), float(np.asarray(gamma).reshape(-1)[0]),
                 np.float32)
    ocol = np.ones((128, 1), ml_dtypes.bfloat16)
    orow = np.ones((1, C), ml_dtypes.bfloat16)
    ident = np.eye(128, dtype=ml_dtypes.bfloat16)
    maps = []
    for i in range(B):
        maps.append({
            "xt": xt_h[i], "xh": xb_h[i],
            "wqt": wqt, "wkt": wkt, "wvo": wvo,
            "gamma_col": gc, "ones_col": ocol, "ones_row": orow,
            "ident": ident,
        })
    return maps


def kernel(x, Wq, Wk, Wv, gamma, _trace=False, _trace_kwargs=None):
    nc = _get_nc()
    in_maps = _make_in_maps(np.asarray(x), np.asarray(Wq), np.asarray(Wk),
                            np.asarray(Wv), np.asarray(gamma))
    kwargs = {}
    if _trace:
        kwargs = dict(trace=True, **(_trace_kwargs or {}))
    res = bass_utils.run_bass_kernel_spmd(nc, in_maps,
                                          core_ids=list(range(B)), **kwargs)
    y = np.stack([np.asarray(res.results[i]["y"]).astype(np.float32)
                  .reshape(C, HH, WW) for i in range(B)])
    if _trace:
        kernel._last_result = res
    return y


# revision 21
# speedup vs baseline: 1.0607x; 1.0607x over previous
"""Trainium2 Bass kernel for the contextual channel-attention transformer block.

Contract: kernel(**inputs) takes the FULL unsharded inputs
(x: (8,512,64,64) f32, Wq/Wk/Wv: (512,512) f32, gamma: (1,) f32) and
returns the FULL (8,512,64,64) f32 output.  Internally the batch is
data-parallel across 8 NeuronCores (one batch element per core).

Per-core algorithm (bf16 matmuls, fp32 PSUM accumulation):
  Gx   = X X^T                        (C x C spatial Gram, 128 MMs)
  M3q  = Gx Wq^T, M3k = Gx Wk^T       (32 MMs)
  |Q_c|^2 = colsum(Wq^T o M3q), |K_c|^2 = colsum(Wk^T o M3k)
  rsqrt via magic constant + 2 Newton steps (DVE only, no act tables)
  msc  = M3q o rq (Q-norm folded)     so the softmax reads G^T directly
  G^T  = Wk msc = (Q K^T)^T o rq      (16 MMs)
  cos -> col-max -> temperature -> softmax: free-axis ops on G^T[d, c]
  A^T  = Wv^T Msm^T                   (16 MMs)
  A''  = A + diag((rowsum+eps)/gamma) (residual identity, 1/f folded)
  y    = f o (A'' X)                  (128 MMs; f applied in PSUM copies)
"""

import os
import sys

for _p in ("/opt/trn_rl_repo", "/root/.axon_site/_ro/trn_rl_repo"):
    if os.path.isdir(_p) and _p not in sys.path:
        sys.path.insert(0, _p)

import ml_dtypes
import numpy as np

import concourse.bass as bass
import concourse.tile as tile
from concourse import bacc, bass_utils, mybir

# Problem constants (hardcoded; kernel.py must be self-contained).
B, C, HH, WW = 8, 512, 64, 64
N = HH * WW          # 4096 spatial positions
G = C // 128         # 4 channel groups of 128
N1 = N // 128        # 32 Gram chunks (128 spatial each)
NJ = N // 512        # 8 output chunks (512 spatial each)
EPS = 1e-6
INV_H = 4.0          # 1 / 0.25 temperature
FP32 = mybir.dt.float32
BF16 = mybir.dt.bfloat16
U32 = mybir.dt.uint32
MAGIC = 0x5F3759DF

_CACHE = {}


def _rsqrt(nc, pool, out_ap, in_ap, shape, tag):
    """out = 1/sqrt(in) via magic constant + 2 Newton iterations (DVE)."""
    MUL = mybir.AluOpType.mult
    ADD = mybir.AluOpType.add
    SHR = mybir.AluOpType.logical_shift_right
    SUB = mybir.AluOpType.subtract
    p, f = shape
    sh = pool.tile([p, f], U32, tag=f"{tag}_sh")
    nc.vector.tensor_scalar(sh[:], in_ap.bitcast(U32), 1, None, op0=SHR)
    mg = pool.tile([p, f], U32, tag=f"{tag}_mg")
    nc.vector.memset(mg[:], MAGIC)
    y = pool.tile([p, f], FP32, tag=f"{tag}_y")
    nc.vector.tensor_tensor(y[:].bitcast(U32), mg[:], sh[:], op=SUB)
    for it in range(2):
        a = pool.tile([p, f], FP32, tag=f"{tag}_a{it}")
        nc.vector.tensor_tensor(a[:], y[:], y[:], op=MUL)
        b = pool.tile([p, f], FP32, tag=f"{tag}_b{it}")
        nc.vector.scalar_tensor_tensor(b[:], in_ap, -0.5, a[:],
                                       op0=MUL, op1=MUL)
        c2 = pool.tile([p, f], FP32, tag=f"{tag}_c{it}")
        nc.vector.tensor_scalar(c2[:], b[:], 1.5, None, op0=ADD)
        dst = out_ap if it == 1 else y[:]
        nc.vector.tensor_tensor(dst, y[:], c2[:], op=MUL)


def _build_nc():
    nc = bacc.Bacc("TRN2", target_bir_lowering=False)

    xt_d = nc.dram_tensor("xt", [N, C], BF16, kind="ExternalInput")   # x^T
    xh_d = nc.dram_tensor("xh", [C, N], BF16, kind="ExternalInput")
    wqt_d = nc.dram_tensor("wqt", [C, C], BF16, kind="ExternalInput")  # Wq^T
    wkt_d = nc.dram_tensor("wkt", [C, C], BF16, kind="ExternalInput")  # Wk^T
    wvo_d = nc.dram_tensor("wvo", [C, C], BF16, kind="ExternalInput")  # Wv
    gcol_d = nc.dram_tensor("gamma_col", [128, 1], FP32, kind="ExternalInput")
    ocol_d = nc.dram_tensor("ones_col", [128, 1], BF16, kind="ExternalInput")
    orow_d = nc.dram_tensor("ones_row", [1, C], BF16, kind="ExternalInput")
    ident_d = nc.dram_tensor("ident", [128, 128], BF16, kind="ExternalInput")
    osq_d = nc.dram_tensor("ones_sq", [128, 128], BF16, kind="ExternalInput")
    y_d = nc.dram_tensor("y", [C, N], BF16, kind="ExternalOutput")

    xt_v = xt_d.ap().rearrange("(i p) c -> p i c", p=128)    # [128, N1, C]
    xh_v = xh_d.ap().rearrange("(g p) n -> p g n", p=128)    # [128, G, N]
    wq_v = wqt_d.ap().rearrange("(g p) o -> p g o", p=128)   # [128, G, C]
    wk_v = wkt_d.ap().rearrange("(g p) o -> p g o", p=128)
    wv_v = wvo_d.ap().rearrange("(g p) o -> p g o", p=128)
    y_v = y_d.ap().rearrange("(g p) n -> p g n", p=128)

    MUL = mybir.AluOpType.mult
    ADD = mybir.AluOpType.add
    MIN = mybir.AluOpType.min
    AX = mybir.AxisListType.X
    Exp = mybir.ActivationFunctionType.Exp
    Copy = mybir.ActivationFunctionType.Copy

    with tile.TileContext(nc) as tc:
        with (
            tc.tile_pool(name="consts", bufs=1) as cpool,
            tc.tile_pool(name="weights", bufs=1) as wpool,
            tc.tile_pool(name="xt", bufs=NJ) as xtpool,
            tc.tile_pool(name="xh", bufs=NJ) as xhpool,
            tc.tile_pool(name="gram", bufs=1) as gpool,
            tc.tile_pool(name="small", bufs=2) as spool,
            tc.tile_pool(name="mid", bufs=3) as mpool,
            tc.tile_pool(name="msm", bufs=1) as msmpool,
            tc.tile_pool(name="outs", bufs=4) as opool,
        ):
            # ---- input DMAs: xt on sync (first), everything else scalar --
            # 8 single-chunk tiles first (more parallel streams early),
            # then 6 quads.
            NS = 8
            xt0 = []
            for i in range(NS):
                t = xtpool.tile([128, 1, C], BF16, tag="xt0", bufs=NS,
                                name=f"xt0_{i}")
                nc.sync.dma_start(t[:], xt_v[:, i:i + 1, :])
                xt0.append(t)
            xt_t = []
            for jj in range(2, NJ):
                t = xtpool.tile([128, G, C], BF16, tag="xt", bufs=NJ - 2,
                                name=f"xt{jj}")
                nc.sync.dma_start(t[:], xt_v[:, jj * G:(jj + 1) * G, :])
                xt_t.append(t)

            def xt_chunk(i):
                if i < NS:
                    return xt0[i][:, 0, :]
                return xt_t[i // G - 2][:, i % G, :]

            ones_col = cpool.tile([128, 1], BF16, tag="ones_col")
            nc.scalar.dma_start(ones_col[:], ocol_d.ap())
            ones_row = cpool.tile([1, C], BF16, tag="ones_row")
            nc.scalar.dma_start(ones_row[:], orow_d.ap())
            gamma_c = cpool.tile([128, 1], FP32, tag="gamma_c")
            nc.scalar.dma_start(gamma_c[:], gcol_d.ap())
            ident = cpool.tile([128, 128], BF16, tag="ident")
            nc.scalar.dma_start(ident[:], ident_d.ap())
            ones_sq = cpool.tile([128, 128], BF16, tag="ones_sq")
            nc.scalar.dma_start(ones_sq[:], osq_d.ap())

            wq = wpool.tile([128, G, C], BF16, tag="wq")
            wk = wpool.tile([128, G, C], BF16, tag="wk")
            wv = wpool.tile([128, G, C], BF16, tag="wv")
            nc.scalar.dma_start(wq[:], wq_v)
            nc.scalar.dma_start(wk[:], wk_v)
            nc.scalar.dma_start(wv[:], wv_v)

            xh_t = []
            for j in range(NJ):
                t = xhpool.tile([128, G, 512], BF16, tag="xh", name=f"xh{j}")
                nc.scalar.dma_start(t[:], xh_v[:, :, j * 512:(j + 1) * 512])
                xh_t.append(t)

            # rgamma = 1/gamma (for the 1/f identity diagonal)
            rgamma = spool.tile([128, 1], FP32, tag="rgamma", bufs=1)
            nc.vector.reciprocal(rgamma[:], gamma_c[:])

            # ---- Gx = X X^T  (PSUM-accumulated over 32 spatial chunks) ---
            gx_sb = gpool.tile([128, G, C], BF16, tag="gx_sb")
            with tc.tile_pool(name="psGx", bufs=1, space="PSUM") as psGx:
                gx_ps = [psGx.tile([128, C], FP32, tag="gx", bufs=G,
                                   name=f"gx{cg}") for cg in range(G)]
                for i in range(N1):
                    lhs_t = xt_chunk(i)
                    for cg in range(G):
                        nc.tensor.matmul(gx_ps[cg][:],
                                         lhs_t[:, cg * 128:(cg + 1) * 128],
                                         lhs_t[:],
                                         start=(i == 0), stop=(i == N1 - 1))
                for cg in range(G):
                    eng = nc.scalar.copy if cg % 2 else nc.vector.tensor_copy
                    eng(gx_sb[:, cg, :], gx_ps[cg][:])

            # ---- M3q = Gx Wq^T, M3k = Gx Wk^T ---------------------------
            m3q = gpool.tile([128, G, C], BF16, tag="m3q")
            m3k = gpool.tile([128, G, C], BF16, tag="m3k")
            tqs, tks = [], []
            with tc.tile_pool(name="psM3", bufs=1, space="PSUM") as psM3:
                for cg in range(G):
                    q_ps = psM3.tile([128, C], FP32, tag="m3q", bufs=2,
                                     name=f"m3q{cg}")
                    k_ps = psM3.tile([128, C], FP32, tag="m3k", bufs=2,
                                     name=f"m3k{cg}")
                    for g in range(G):
                        lhs = gx_sb[:, g, cg * 128:(cg + 1) * 128]
                        nc.tensor.matmul(q_ps[:], lhs, wq[:, g, :],
                                         start=(g == 0), stop=(g == G - 1))
                        nc.tensor.matmul(k_ps[:], lhs, wk[:, g, :],
                                         start=(g == 0), stop=(g == G - 1))
                    nc.scalar.copy(m3q[:, cg, :], q_ps[:])
                    nc.scalar.copy(m3k[:, cg, :], k_ps[:])
                    # tq = Wq^T o M3q, tk = Wk^T o M3k (row-blocks)
                    tq = mpool.tile([128, C], BF16, tag="tq", bufs=2,
                                    name=f"tq{cg}")
                    nc.vector.tensor_tensor(tq[:], wq[:, cg, :],
                                            m3q[:, cg, :], op=MUL)
                    tqs.append(tq)
                    tk = mpool.tile([128, C], BF16, tag="tk", bufs=2,
                                    name=f"tk{cg}")
                    nc.gpsimd.tensor_tensor(tk[:], wk[:, cg, :],
                                            m3k[:, cg, :], op=MUL)
                    tks.append(tk)

            msc = gpool.tile([128, G, C], BF16, tag="msc")
            rk_all = spool.tile([128, G], FP32, tag="rk_all", bufs=1)
            rq_all = spool.tile([128, G], FP32, tag="rq_all", bufs=1)
            with tc.tile_pool(name="psN", bufs=1, space="PSUM") as psN:
                # |Q_c|^2, |K_d|^2 as columns (32 small MMs), batch rsqrt
                sqq_ps = [psN.tile([128, 1], FP32, tag="sqq", bufs=G,
                                   name=f"sqq{d}") for d in range(G)]
                sqk_ps = [psN.tile([128, 1], FP32, tag="sqk", bufs=G,
                                   name=f"sqk{d}") for d in range(G)]
                for g in range(G):
                    for dg in range(G):
                        nc.tensor.matmul(sqq_ps[dg][:],
                                         tqs[g][:, dg * 128:(dg + 1) * 128],
                                         ones_col[:],
                                         start=(g == 0), stop=(g == G - 1))
                        nc.tensor.matmul(sqk_ps[dg][:],
                                         tks[g][:, dg * 128:(dg + 1) * 128],
                                         ones_col[:],
                                         start=(g == 0), stop=(g == G - 1))
                sqq_sb = spool.tile([128, G], FP32, tag="sqq_sb", bufs=1)
                sqk_sb = spool.tile([128, G], FP32, tag="sqk_sb", bufs=1)
                for dg in range(G):
                    nc.vector.tensor_copy(sqq_sb[:, dg:dg + 1],
                                          sqq_ps[dg][:])
                    nc.scalar.copy(sqk_sb[:, dg:dg + 1], sqk_ps[dg][:])
                _rsqrt(nc, spool, rq_all[:], sqq_sb[:], (128, G), "rq")
                _rsqrt(nc, spool, rk_all[:], sqk_sb[:], (128, G), "rk")

            with tc.tile_pool(name="psN2", bufs=1, space="PSUM") as psN2:
                # bq block cg: broadcast rq[cg-block] along partitions via
                # replicate-then-transpose (V1 = ones o rq_col; V1^T @ I)
                bq_b = []
                for cg in range(G):
                    v1 = spool.tile([128, 128], BF16, tag="v1", bufs=2,
                                    name=f"v1_{cg}")
                    nc.vector.tensor_scalar(v1[:], ones_sq[:],
                                            rq_all[:, cg:cg + 1], None,
                                            op0=MUL)
                    bqp = psN2.tile([128, 128], FP32, tag="bq", bufs=G,
                                    name=f"bq{cg}")
                    nc.tensor.matmul(bqp[:], v1[:], ident[:],
                                     start=True, stop=True)
                    bq_b.append(bqp)
                # msc = m3q o rq (column scale, blockwise; bq read from PSUM)
                for g in range(G):
                    for cg in range(G):
                        nc.vector.tensor_tensor(
                            msc[:, g, cg * 128:(cg + 1) * 128],
                            m3q[:, g, cg * 128:(cg + 1) * 128],
                            bq_b[cg][:], op=MUL)

            msm = msmpool.tile([128, G, C], BF16, tag="msm")
            at_sb = gpool.tile([128, G, C], BF16, tag="at_sb")
            fcols = []
            with tc.tile_pool(name="psB", bufs=1, space="PSUM") as psB:
                # ---- G^T per d-group + softmax + A^T ---------------------
                at_ps = [psB.tile([128, C], FP32, tag="at", bufs=G,
                                  name=f"at{eg}") for eg in range(G)]
                for dg in range(G):
                    rk = rk_all[:, dg:dg + 1]
                    g_ps = psB.tile([128, C], FP32, tag="g_ps", bufs=2,
                                    name=f"g_ps{dg}")
                    for g in range(G):
                        nc.tensor.matmul(g_ps[:],
                                         wk[:, g, dg * 128:(dg + 1) * 128],
                                         msc[:, g, :],
                                         start=(g == 0), stop=(g == G - 1))
                    mn = spool.tile([128, 1], FP32, tag="mn")
                    nc.vector.tensor_reduce(mn[:], g_ps[:], axis=AX, op=MIN)
                    # den = 1 + eps - min_c(cos) = 1 + eps - mn * rk
                    mr = spool.tile([128, 1], FP32, tag="mr")
                    nc.vector.tensor_tensor(mr[:], mn[:], rk, op=MUL)
                    den = spool.tile([128, 1], FP32, tag="den")
                    nc.vector.tensor_scalar(den[:], mr[:], -1.0, 1.0 + EPS,
                                            op0=MUL, op1=ADD)
                    r = spool.tile([128, 1], FP32, tag="r")
                    nc.vector.reciprocal(r[:], den[:])
                    # exp((INV_H*r*rk) * gps + (1 - INV_H*r))
                    sv = spool.tile([128, 1], FP32, tag="sv")
                    nc.vector.scalar_tensor_tensor(sv[:], r[:], INV_H, rk,
                                                   op0=MUL, op1=MUL)
                    bv = spool.tile([128, 1], FP32, tag="bv")
                    nc.vector.tensor_scalar(bv[:], r[:], -INV_H, 1.0,
                                            op0=MUL, op1=ADD)
                    e = mpool.tile([128, C], BF16, tag="e")
                    se = spool.tile([128, 1], FP32, tag="se")
                    nc.scalar.activation(e[:], g_ps[:], Exp,
                                         bias=bv[:], scale=sv[:],
                                         accum_out=se[:])
                    rd = spool.tile([128, 1], FP32, tag="rd")
                    nc.vector.reciprocal(rd[:], se[:])
                    nc.vector.tensor_scalar(msm[:, dg, :], e[:], rd[:], None,
                                            op0=MUL)
                    # A^T accumulation over d
                    for eg in range(G):
                        nc.tensor.matmul(at_ps[eg][:],
                                         wv[:, dg, eg * 128:(eg + 1) * 128],
                                         msm[:, dg, :],
                                         start=(dg == 0), stop=(dg == G - 1))

                # row-L1 sums; f = gamma/(s+eps) fp32 cols; A'' diag = 1/f
                s_list = []
                for cg in range(G):
                    s_ps = psB.tile([128, 1], FP32, tag="g_ps", bufs=2,
                                    name=f"s_ps{cg}")
                    for dg in range(G):
                        nc.tensor.matmul(
                            s_ps[:],
                            msm[:, dg, cg * 128:(cg + 1) * 128],
                            ones_col[:], start=(dg == 0), stop=(dg == G - 1))
                    s_list.append(s_ps)
                for eg in range(G):
                    nc.scalar.copy(at_sb[:, eg, :], at_ps[eg][:])
                for cg in range(G):
                    seps = spool.tile([128, 1], FP32, tag="seps")
                    nc.vector.tensor_scalar(seps[:], s_list[cg][:],
                                            EPS, None, op0=ADD)
                    rs = spool.tile([128, 1], FP32, tag="rs")
                    nc.vector.reciprocal(rs[:], seps[:])
                    f = spool.tile([128, 1], FP32, tag="f", bufs=G,
                                   name=f"f{cg}")
                    nc.vector.tensor_tensor(f[:], rs[:], gamma_c[:], op=MUL)
                    fcols.append(f)
                    finv = spool.tile([128, 1], FP32, tag="finv")
                    nc.vector.tensor_tensor(finv[:], seps[:], rgamma[:],
                                            op=MUL)
                    di = spool.tile([128, 128], BF16, tag="di")
                    nc.vector.tensor_scalar(di[:], ident[:], finv[:], None,
                                            op0=MUL)
                    blk = at_sb[:, cg, cg * 128:(cg + 1) * 128]
                    nc.vector.tensor_tensor(blk, blk, di[:], op=ADD)

            # ---- phase 2: y = f o (A'' X), store bf16 --------------------
            with tc.tile_pool(name="ps2", bufs=1, space="PSUM") as ps2:
                for j in range(NJ):
                    for cg in range(G):
                        o_ps = ps2.tile([128, 512], FP32, tag="o_ps", bufs=8,
                                        name=f"o_ps{j}_{cg}")
                        for eg in range(G):
                            nc.tensor.matmul(
                                o_ps[:], at_sb[:, eg, cg * 128:(cg + 1) * 128],
                                xh_t[j][:, eg, :],
                                start=(eg == 0), stop=(eg == G - 1))
                        ofin = opool.tile([128, 512], BF16, tag="ofin",
                                          bufs=8, name=f"ofin{j}_{cg}")
                        k = j * G + cg
                        if k % 2:
                            nc.scalar.activation(ofin[:], o_ps[:], Copy,
                                                 scale=fcols[cg][:])
                        else:
                            nc.vector.tensor_scalar(ofin[:], o_ps[:],
                                                    fcols[cg][:], None,
                                                    op0=MUL)
                        nc.sync.dma_start(
                            y_v[:, cg, j * 512:(j + 1) * 512], ofin[:])

    nc.compile()
    return nc


def _get_nc():
    if "nc" not in _CACHE:
        _CACHE["nc"] = _build_nc()
    return _CACHE["nc"]


def _make_in_maps(x, Wq, Wk, Wv, gamma):
    xb_h = np.ascontiguousarray(
        x.reshape(B, C, N)).astype(ml_dtypes.bfloat16)
    xt_h = np.ascontiguousarray(xb_h.transpose(0, 2, 1))
    wqt = np.ascontiguousarray(Wq.T).astype(ml_dtypes.bfloat16)
    wkt = np.ascontiguousarray(Wk.T).astype(ml_dtypes.bfloat16)
    wvo = np.ascontiguousarray(Wv).astype(ml_dtypes.bfloat16)
    gc = np.full((128, 1), float(np.asarray(gamma).reshape(-1)[0]),
                 np.float32)
    ocol = np.ones((128, 1), ml_dtypes.bfloat16)
    orow = np.ones((1, C), ml_dtypes.bfloat16)
    ident = np.eye(128, dtype=ml_dtypes.bfloat16)
    osq = np.ones((128, 128), dtype=ml_dtypes.bfloat16)
    maps = []
    for i in range(B):
        maps.append({
            "xt": xt_h[i], "xh": xb_h[i],
            "wqt": wqt, "wkt": wkt, "wvo": wvo,
            "gamma_col": gc, "ones_col": ocol, "ones_row": orow,
            "ident": ident, "ones_sq": osq,
        })
    return maps


def kernel(x, Wq, Wk, Wv, gamma, _trace=False, _trace_kwargs=None):
    nc = _get_nc()
    in_maps = _make_in_maps(np.asarray(x), np.asarray(Wq), np.asarray(Wk),
                            np.asarray(Wv), np.asarray(gamma))
    kwargs = {}
    if _trace:
        kwargs = dict(trace=True, **(_trace_kwargs or {}))
    res = bass_utils.run_bass_kernel_spmd(nc, in_maps,
                                          core_ids=list(range(B)), **kwargs)
    y = np.stack([np.asarray(res.results[i]["y"]).astype(np.float32)
                  .reshape(C, HH, WW) for i in range(B)])
    if _trace:
        kernel._last_result = res
    return y
